# revision 15
# baseline (speedup 1.0000x reference)
"""Trainium2 Bass kernel for nn_Attention_layer (dense_transformer).

One batch element per NeuronCore (8 cores). hw-major layout: pos = hw*64 + d
(host pre-permutes x and un-permutes the output), so each 16-hw chunk of
1024 positions is independent through the attention core and the kernel runs
as a software-pipelined 16-chunk stream (projections of chunk g+1 are
emitted between the attention phases of chunk g to keep the PE dense).

Per chunk g (16 hw, cols = g*1024 ... +1024):
  A: x' chunk DMA (one rearranged [128,2,1024] load)
     k = Wk x'   2x[128,512] psum -> ACT drain (+bias -Wk bo)      -> bf16
     q = Wq x'   likewise (+bias bq - Wq bo)
     vT direct:  lhsT = x'-pair [128,128], rhs = Wv k-tiles -> [u*64+d, s]
  B: scoresT[j,i] = q_hw^T k_hw per hw (16 MMs into one psum bank)
     aT = exp(scoresT/sqrt(S)) (ACT) -> reduce_sum/reciprocal (DVE)
     normalize aT per hw-pair (gpsimd)
     a = aT^T per hw (16 PE transposes, one bf16 psum bank) -> DVE drain
     att_hw = vT_hw^T a_hw; psum banks split by hw PARITY (the PE cannot
       alternate the K base partition inside one bank) -> ACT (+bv - Wv bo)
     out = Wo att (4 MMs on a pos-ordered strided view) -> DVE add x' -> DMA

Bias algebra: host sends x' = x + bo. Spurious W*bo terms cancel via
per-partition drain biases; bk drops exactly (softmax per-j const); bv rides
the att drain (softmax columns sum to 1); bo lands via the residual.
All bf16 inputs / f32 accum (fp8 DoubleRow fails the 2e-2 gate numerically).
"""

import numpy as np
import ml_dtypes

import concourse.bacc as bacc
import concourse.tile as tile
from concourse import mybir
from concourse.bass_utils import run_bass_kernel_spmd

F32 = mybir.dt.float32
BF16 = mybir.dt.bfloat16
AF = mybir.ActivationFunctionType

B, C, S, D, H, W = 8, 256, 128, 64, 16, 16
HW = H * W              # 256
NPOS = D * HW           # 16384, pos = hw*64 + d  (hw-major)
NCH = 16                # chunks
CHW = HW // NCH         # 16 hw per chunk
CCOL = CHW * D          # 1024 cols per chunk
SCALE = float(1.0 / np.sqrt(np.float32(S)))

CFG = {
    "loop_n": 1,   # on-device repeats of the whole body (timing)
    "sp": True,    # software-pipelined emission (A of g+1 before B of g)
    "trace": False,
}

_CACHE = {}


def _emit(nc, tc, io, ctx):
    x_d, wk_d, wq_d, wv_d, wo_d, bkc, bqc, bvc, ident, out_d = io

    const = ctx.enter_context(tc.tile_pool(name="const", bufs=1))
    xin = ctx.enter_context(tc.tile_pool(name="xin", bufs=4))
    kqv = ctx.enter_context(tc.tile_pool(name="kqv", bufs=8))
    vtp = ctx.enter_context(tc.tile_pool(name="vtp", bufs=3))
    smx = ctx.enter_context(tc.tile_pool(name="smx", bufs=6))
    att = ctx.enter_context(tc.tile_pool(name="attp", bufs=3))
    outp = ctx.enter_context(tc.tile_pool(name="outp", bufs=3))
    # PSUM (8 banks of [128,512]f32): proj/vT ring 3 + scores/a 2 + att/out 3
    pp_h = ctx.enter_context(tc.tile_pool(name="pp_h", bufs=3, space="PSUM"))
    pp_sa = ctx.enter_context(tc.tile_pool(name="pp_sa", bufs=2, space="PSUM"))
    pp_ao = ctx.enter_context(tc.tile_pool(name="pp_ao", bufs=3, space="PSUM"))

    # ---- constants ------------------------------------------------------
    id_sb = const.tile([128, 128], BF16, tag="ident")
    nc.sync.dma_start(id_sb[:], ident[:])
    w_sb = {}
    for nm, t in (("wk", wk_d), ("wq", wq_d), ("wv", wv_d), ("wo", wo_d)):
        for h in range(2):
            w_sb[nm, h] = const.tile([128, 128], BF16, tag=f"w_{nm}{h}",
                                     name=f"w_{nm}{h}")
            if nm == "wo":  # wo_d is [S, C]: column halves
                nc.sync.dma_start(w_sb[nm, h][:], t[:, h * 128:(h + 1) * 128])
            else:           # [C, S]: row halves (K-tiles)
                nc.sync.dma_start(w_sb[nm, h][:], t[h * 128:(h + 1) * 128, :])
    b_sb = {}
    for nm, t in (("bk", bkc), ("bq", bqc), ("bv", bvc)):
        b_sb[nm] = const.tile([128, 1], F32, tag=f"b_{nm}", name=f"b_{nm}")
        nc.sync.dma_start(b_sb[nm][:], t[:])

    loop_cm = tc.For_i(0, CFG["loop_n"], 1) if CFG["loop_n"] > 1 else None
    if loop_cm is not None:
        ctx.enter_context(loop_cm)

    stA = {}

    def phase_a(g):
        cs = slice(g * CCOL, (g + 1) * CCOL)
        # ---- load x chunk (x' = x + bo, bf16), both c-halves, one DMA ---
        xc = xin.tile([128, 2, CCOL], BF16, tag="xc", name=f"xc{g}")
        nc.sync.dma_start(xc[:],
                          x_d[:, cs].rearrange("(h p) n -> p h n", h=2))

        # ---- projections k, q (two [128,512] psum halves each) ----------
        sb = {}
        for nm, bias in (("wk", "bk"), ("wq", "bq")):
            dst = kqv.tile([128, CCOL], BF16, tag="kqv", name=f"s{nm}{g}")
            sb[nm] = dst
            for cg in range(2):
                sl = slice(cg * 512, (cg + 1) * 512)
                ps = pp_h.tile([128, 512], F32, tag="h", name=f"p{nm}{g}{cg}")
                nc.tensor.matmul(ps[:], w_sb[nm, 0][:], xc[:, 0, sl],
                                 start=True, stop=False)
                nc.tensor.matmul(ps[:], w_sb[nm, 1][:], xc[:, 1, sl],
                                 start=False, stop=True)
                nc.scalar.activation(dst[:, sl], ps[:], AF.Identity,
                                     bias=b_sb[bias][:], scale=1.0)

        # ---- vT direct: lhsT = x-pair (stationary), rhs = Wv k-tiles ----
        # out [128(u*64+d), 128(s)] per pair, accumulated over c-halves
        vt = vtp.tile([128, CCOL], BF16, tag="vt", name=f"vt{g}")
        for half in range(2):
            ps = pp_h.tile([128, 512], F32, tag="h", name=f"pvt{g}{half}")
            for pp in range(4):
                p = half * 4 + pp
                xsl = xc[:, :, p * 128:(p + 1) * 128]
                nc.tensor.matmul(ps[:, pp * 128:(pp + 1) * 128],
                                 xsl[:, 0, :], w_sb["wv", 0][:],
                                 start=True, stop=False,
                                 skip_group_check=True)
                nc.tensor.matmul(ps[:, pp * 128:(pp + 1) * 128],
                                 xsl[:, 1, :], w_sb["wv", 1][:],
                                 start=False, stop=True,
                                 skip_group_check=True)
            nc.vector.tensor_copy(vt[:, half * 512:(half + 1) * 512], ps[:])
        stA[g] = (xc, sb, vt)

    def phase_b(g):
        cs = slice(g * CCOL, (g + 1) * CCOL)
        xc, sb, vt = stA.pop(g)

        # ---- scoresT = q_hw^T k_hw : [64(j), 64(i)] per hw --------------
        # pack: partitions (hw%2)*64, free (hw//2)*64  -> [128, 512]
        ps_s = pp_sa.tile([128, 512], F32, tag="sa", name=f"sc{g}")
        for hw in range(CHW):
            u, p = hw % 2, hw // 2
            nc.tensor.matmul(
                ps_s[u * 64:(u + 1) * 64, p * 64:(p + 1) * 64],
                sb["wq"][:, hw * 64:(hw + 1) * 64],
                sb["wk"][:, hw * 64:(hw + 1) * 64],
                start=True, stop=True, skip_group_check=True)

        # ---- softmax over i (free axis), skip-max -----------------------
        aT = smx.tile([128, 512], BF16, tag="aT", name=f"aT{g}")
        nc.scalar.activation(aT[:], ps_s[:], AF.Exp, scale=SCALE)
        denom = smx.tile([128, 8], F32, tag="dn", name=f"dn{g}")
        rcp = smx.tile([128, 8], F32, tag="rc", name=f"rc{g}")
        nc.vector.reduce_sum(
            out=denom[:],
            in_=aT[:].rearrange("p (i f) -> p i f", i=8),
            axis=mybir.AxisListType.X)
        nc.vector.reciprocal(rcp[:], denom[:])
        for p in range(8):
            nc.gpsimd.tensor_scalar_mul(aT[:, p * 64:(p + 1) * 64],
                                        aT[:, p * 64:(p + 1) * 64],
                                        rcp[:, p:p + 1])

        # ---- a = aT^T per hw: [64(j),64(i)] at part u*64 -> [64(i),64(j)]
        ps_a = pp_sa.tile([128, 512], BF16, tag="sa", name=f"a{g}")
        for hw in range(CHW):
            u, p = hw % 2, hw // 2
            nc.tensor.matmul(
                ps_a[u * 64:(u + 1) * 64, p * 64:(p + 1) * 64],
                aT[u * 64:(u + 1) * 64, p * 64:(p + 1) * 64],
                id_sb[u * 64:(u + 1) * 64, u * 64:(u + 1) * 64],
                is_transpose=True, start=True, stop=True,
                skip_group_check=True)
        a_sb = smx.tile([128, 512], BF16, tag="as", name=f"as{g}")
        nc.vector.tensor_copy(a_sb[:], ps_a[:])

        # ---- att_hw = vT_hw^T a_hw : [128(s), 64(j)] per hw -------------
        # psum banks split by hw parity (no K-base alternation in a bank);
        # att_sb is parity-major: cols u*512 + p*64 hold att of hw = 2p+u.
        att_sb = att.tile([128, CCOL], BF16, tag="att", name=f"att{g}")
        for u in range(2):
            ps_t = pp_ao.tile([128, 512], F32, tag="ao", name=f"at{g}{u}")
            for p in range(8):
                nc.tensor.matmul(
                    ps_t[:, p * 64:(p + 1) * 64],
                    vt[u * 64:(u + 1) * 64, p * 128:(p + 1) * 128],
                    a_sb[u * 64:(u + 1) * 64, p * 64:(p + 1) * 64],
                    start=(p == 0), stop=(p == 7), skip_group_check=True)
            nc.scalar.activation(att_sb[:, u * 512:(u + 1) * 512],
                                 ps_t[:], AF.Identity,
                                 bias=b_sb["bv"][:], scale=1.0)
        # pos-ordered view of the parity-major att columns
        att_pos = att_sb[:].rearrange("c (u p d) -> c p u d", u=2, d=D)

        # ---- out = Wo att + x' (residual includes bo) -------------------
        ot = outp.tile([128, 2, CCOL], BF16, tag="out", name=f"o{g}")
        for h in range(2):
            for cg in range(2):
                sl = slice(cg * 512, (cg + 1) * 512)
                ps = pp_ao.tile([128, 512], F32, tag="ao", name=f"op{g}{h}{cg}")
                nc.tensor.matmul(ps[:], w_sb["wo", h][:],
                                 att_pos[:, 4 * cg:4 * cg + 4],
                                 start=True, stop=True)
                nc.vector.tensor_add(ot[:, h, sl], ps[:], xc[:, h, sl])
        nc.sync.dma_start(out_d[:, cs].rearrange("(h p) n -> p h n", h=2),
                          ot[:])

    if CFG["sp"]:
        for g in range(NCH + 1):
            if g < NCH:
                phase_a(g)
            if g >= 1:
                phase_b(g - 1)
    else:
        for g in range(NCH):
            phase_a(g)
            phase_b(g)


def build():
    key = tuple(sorted((k, v) for k, v in CFG.items() if k != "trace"))
    if key in _CACHE:
        return _CACHE[key]
    nc = bacc.Bacc("TRN2", target_bir_lowering=False, debug=False,
                   num_devices=8)
    x_d = nc.dram_tensor("x", [C, NPOS], BF16, kind="ExternalInput")
    wk_d = nc.dram_tensor("wk", [C, S], BF16, kind="ExternalInput")
    wq_d = nc.dram_tensor("wq", [C, S], BF16, kind="ExternalInput")
    wv_d = nc.dram_tensor("wv", [C, S], BF16, kind="ExternalInput")
    wo_d = nc.dram_tensor("wo", [S, C], BF16, kind="ExternalInput")
    bkc = nc.dram_tensor("bkc", [S, 1], F32, kind="ExternalInput")
    bqc = nc.dram_tensor("bqc", [S, 1], F32, kind="ExternalInput")
    bvc = nc.dram_tensor("bvc", [S, 1], F32, kind="ExternalInput")
    ident = nc.dram_tensor("ident", [128, 128], BF16, kind="ExternalInput")
    out_d = nc.dram_tensor("out", [C, NPOS], BF16, kind="ExternalOutput")
    from contextlib import ExitStack
    with tile.TileContext(nc) as tc, ExitStack() as ctx:
        _emit(nc, tc,
              (x_d, wk_d, wq_d, wv_d, wo_d, bkc, bqc, bvc, ident, out_d),
              ctx)
    nc.compile()
    _CACHE[key] = nc
    return nc


def make_in_maps(x, wk, bk, wq, bq, wv, bv, wo, bo):
    bf = ml_dtypes.bfloat16
    x = np.asarray(x, np.float32)
    wk = np.asarray(wk, np.float32)
    wq = np.asarray(wq, np.float32)
    wv = np.asarray(wv, np.float32)
    wo = np.asarray(wo, np.float32)
    bk = np.asarray(bk, np.float32)
    bq = np.asarray(bq, np.float32)
    bv = np.asarray(bv, np.float32)
    bo = np.asarray(bo, np.float32)
    # hw-major positions + fold bo into the x tensor (residual carries bo)
    xp = np.ascontiguousarray(
        x.reshape(B, C, D, HW).transpose(0, 1, 3, 2)).reshape(B, C, NPOS)
    xp = xp + bo[None, :, None]
    com = {
        "wk": np.ascontiguousarray(wk.T).astype(bf),   # [C, S] lhsT
        "wq": np.ascontiguousarray(wq.T).astype(bf),
        "wv": np.ascontiguousarray(wv.T).astype(bf),
        "wo": np.ascontiguousarray(wo.T).astype(bf),   # [S, C] lhsT
        "bkc": (-wk @ bo).reshape(S, 1),
        "bqc": (bq - wq @ bo).reshape(S, 1),
        "bvc": (bv - wv @ bo).reshape(S, 1),
        "ident": np.eye(128, dtype=bf),
    }
    return [dict(com, x=xp[b].astype(bf)) for b in range(B)]


def postprocess(stacked):
    """[B, C, NPOS] hw-major bf16/f32 -> [B, C, D, H, W] f32."""
    out = np.asarray(stacked, np.float32).reshape(B, C, HW, D)
    return np.ascontiguousarray(out.transpose(0, 1, 3, 2)).reshape(
        B, C, D, H, W)


def run(x, wk, bk, wq, bq, wv, bv, wo, bo, **kw):
    nc = build()
    maps = make_in_maps(x, wk, bk, wq, bq, wv, bv, wo, bo)
    res = run_bass_kernel_spmd(nc, maps, core_ids=list(range(B)), **kw)
    out = np.stack([np.asarray(r["out"]) for r in res.results])
    return postprocess(out), res


def kernel(x, wk, bk, wq, bq, wv, bv, wo, bo):
    out, _ = run(x, wk, bk, wq, bq, wv, bv, wo, bo)
    return out
